# revision 58
# baseline (speedup 1.0000x reference)
"""HDMNet (BiMap -> LogEig -> Linear) Trainium2 kernel, 8-core data-parallel.

Math: y_b = W^T x_b W (30x30 SPD), logm(y_b) approximated by a degree-9
Chebyshev polynomial in s = alpha*y + beta*I evaluated with a
Paterson-Stockmeyer block scheme:
    p(s) = q0(s) + T3'(s)*q1(s) + T3'(s)^2*q2'(s)
Per-item 30x30 products run as block-diagonal matmuls (4 items per
128x128 FWL-eligible stationary, built by strided scatter copies on
Scalar/DVE); scalar-coefficient combinations accumulate into the same
PSUM groups as c*I-stationary matmuls; q0's constant term is folded
into the host-side bias.

Layout: items are packed 4-per-partition-group (row groups 32r) and
32-columns-per-slot (30 data + 2 zero pad), so every PSUM region written
is a full 32-row/32-col block and the zero pads propagate exactly
(seeded by the zero-padded W stationary).  One item matrix lives at
[32r : 32r+30, 32s : 32s+30].

BiMap phase 1 uses per-item x-stationaries sliced 128 columns wide so
fast-weight-load engages (~27ns vs 80ns per load); the overhanging 35
columns read the next item's data / zero pad and land in PSUM rows
93..127, which are never read.  The linear layer interleaves its four
row-group accumulation chains across four separate PSUM tiles (chains
must never interleave inside one tile - that wedges the HW).

Dataflow: depth-4 software pipeline (A-phase(t) | T-recurrence(t-1) |
first Horner level(t-2) | second level + output(t-3)) so every engine
FIFO holds ready work; x arrives via SWDGE DMA (spreads across all 16
SDMA engines), with chunk 0/1 quartered onto the HWDGE rings for a fast
start.

Sharding: batch 8192 -> 1024 per NeuronCore; W / lin_w replicated.
"""
import os
import numpy as np

NCORES = 8
B = 8192
DIM, K, CLS = 93, 30, 117
KP = 32                     # padded slot pitch
CHUNK = 64
SLOTS = CHUNK // 4          # 16 slots of 4 stacked items
SW = SLOTS * KP             # 512 cols per chunk-state tile (1 PSUM bank)
QUART = 16 * 93             # 1488 cols per chunk-quarter DMA slab
A_LO, A_HI = 0.076, 3.51

# Chebyshev-basis constants, order:
# [(2,0)..(2,3), (1,0)..(1,2), (0,0)..(0,2)]  (level i, Cheb index k)
CDEV = [
    -0.015716552734375,
    0.0946044921875,
    0.0018758773803710938,
    0.0304412841796875,
    0.255859375,
    -0.202880859375,
    0.131591796875,
    0.18896484375,
    1.35546875,
    -0.395751953125,
]

LAST_EXEC_TIME_NS = None


def _host_consts(W, lin_w, alpha, beta):
    f16 = np.float16
    wt = np.zeros((DIM, KP), f16)
    wt[:, :K] = (np.sqrt(2.0 * alpha) * W).astype(f16)          # [93,32]

    # stacked identity pattern [128, SW]: 2*I at each (group, slot)
    idp2 = np.zeros((128, SW), np.float32)
    eye2 = 2.0 * np.eye(K, dtype=np.float32)
    for r in range(4):
        for s in range(SLOTS):
            idp2[32 * r:32 * r + K, KP * s:KP * s + K] = eye2
    bet2 = (beta * idp2).astype(np.float32)                     # 2*beta*I stacked
    idp2_16 = idp2.astype(f16)

    # constant-diagonal stationaries [128, 10*128] for the coefficient
    # matmuls (movings are [idp2, u1, u2] per level); index 7 (q0's
    # constant term) is unused on-device -- it is folded into the host
    # bias as CDEV[7]*lin_w@vec(I)
    cd = np.zeros((128, 10 * 128), f16)
    i128 = np.eye(128, dtype=np.float32)
    for j, c in enumerate(CDEV):
        cd[:, j * 128:(j + 1) * 128] = (c * i128).astype(f16)
    qc = (CDEV[0] * idp2).astype(f16)            # A3 chain seed

    # linear weights banked, CLS padded to 128: lw[32r+q, p*128+cls]
    lw = np.zeros((128, K * 128), f16)
    lwr = lin_w.reshape(CLS, K, K)          # [cls, p, q]
    blk = np.zeros((K, K * 128), np.float32)
    for p in range(K):
        blk[:, p * 128:p * 128 + CLS] = lwr[:, p, :].T          # [q, cls]
    for r in range(4):
        lw[32 * r:32 * r + K, :] = blk.astype(f16)
    return wt, idp2_16, bet2, cd, qc, lw


def _run(x, W, lin_w, bpc):
    import concourse.bass as bass
    import concourse.bacc as bacc
    import concourse.mybir as mybir
    from concourse.tile import TileContext
    from concourse.bass_utils import run_bass_kernel_spmd

    f16, f32 = mybir.dt.float16, mybir.dt.float32
    MULT, ADD = mybir.AluOpType.mult, mybir.AluOpType.add
    nchunk = bpc // CHUNK
    alpha = 2.0 / (A_HI - A_LO)
    beta2 = -2.0 * (A_HI + A_LO) / (A_HI - A_LO)   # 2*beta

    nc = bacc.Bacc()
    xt_d = nc.dram_tensor("xt", [nchunk * 4 * DIM, QUART], f16,
                          kind="ExternalInput")
    wt_d = nc.dram_tensor("wt", [DIM, KP], f16, kind="ExternalInput")
    idp2_d = nc.dram_tensor("idp2", [128, SW], f16, kind="ExternalInput")
    bet2_d = nc.dram_tensor("bet2", [128, SW], f32, kind="ExternalInput")
    cd_d = nc.dram_tensor("cd", [128, 10 * 128], f16, kind="ExternalInput")
    qc_d = nc.dram_tensor("qc", [128, SW], f16, kind="ExternalInput")
    lw_d = nc.dram_tensor("lw", [128, K * 128], f16, kind="ExternalInput")
    out_d = nc.dram_tensor("out", [CLS, bpc], f32, kind="ExternalOutput")

    XW = CHUNK * DIM           # 5952 data cols per xin tile
    XPAD = XW + 35             # room for the last item's 128-col slice

    with TileContext(nc) as tc:
        with tc.sbuf_pool(name="cpool", bufs=1) as cpool, \
             tc.sbuf_pool(name="xpool", bufs=1) as xpool, \
             tc.sbuf_pool(name="hpool", bufs=4) as hpool, \
             tc.sbuf_pool(name="upool", bufs=3) as upool, \
             tc.sbuf_pool(name="spool", bufs=1) as spool:

            wt_sb = cpool.tile([DIM, KP], f16, name="wt_sb")
            nc.sync.dma_start(out=wt_sb[:], in_=wt_d[:])
            idp2_sb = cpool.tile([128, SW], f16, name="idp2_sb")
            bet2_sb = cpool.tile([128, SW], f32, name="bet2_sb")
            cd_sb = cpool.tile([128, 10 * 128], f16, name="cd_sb")
            qc_sb = cpool.tile([128, SW], f16, name="qc_sb")
            lw_sb = cpool.tile([128, K * 128], f16, name="lw_sb")

            def load_consts():
                # small consts on the scalar ring; big ones (cd/lw) via
                # SWDGE, drained after the startup chunks
                nc.scalar.dma_start(out=bet2_sb[:], in_=bet2_d[:])
                nc.scalar.dma_start(out=idp2_sb[:], in_=idp2_d[:])
                nc.scalar.dma_start(out=qc_sb[:], in_=qc_d[:])
                nc.gpsimd.dma_start(out=cd_sb[:], in_=cd_d[:])
                nc.gpsimd.dma_start(out=lw_sb[:], in_=lw_d[:])

            NXBUF = 5
            xins = [xpool.tile([DIM, XPAD], f16, tag=f"xin{i}",
                               name=f"xin{i}") for i in range(NXBUF)]
            for t in xins:
                nc.vector.memset(t[:, XW:], 0.0)

            # p-major: lg3[z, p*(nchunk*SLOTS) + cc*SLOTS + s]
            # block-diagonal stationaries (zeros persist outside the
            # 30x30 data blocks; scatters rewrite only those)
            sbd1s = [spool.tile([128, SLOTS * 128], f16, name=f"sbd1_{i}")
                     for i in range(2)]
            sbd3s = [spool.tile([128, SLOTS * 128], f16, name=f"sbd3_{i}")
                     for i in range(3)]
            for i, tl in enumerate(sbd1s + sbd3s):
                (nc.vector if i % 2 == 0 else nc.gpsimd).memset(tl[:], 0.0)

            lg3 = spool.tile([128, K * nchunk * SLOTS], f16, name="lg3")
            outsb = spool.tile([CLS, bpc], f32, name="outsb")

            with tc.psum_pool(name="psA", bufs=2) as psA_pool, \
                 tc.psum_pool(name="psS", bufs=1) as psS_pool, \
                 tc.psum_pool(name="psB", bufs=2) as psB_pool, \
                 tc.psum_pool(name="psQ", bufs=2) as psQ_pool, \
                 tc.psum_pool(name="psC", bufs=1) as psC_pool:

                def fetch_x(cc):
                    if cc < 2:
                        # startup: quarter-DMAs on the idle HWDGE rings so
                        # chunk 0's first slots are usable within ~10us
                        for q in range(4):
                            eng = nc.sync if cc == 0 else nc.scalar
                            r0 = (cc * 4 + q) * DIM
                            eng.dma_start(
                                out=xins[cc][:, q * QUART:(q + 1) * QUART],
                                in_=xt_d[r0:r0 + DIM, :])
                        return
                    # steady state via SWDGE (gpsimd): spreads each
                    # transfer across all 16 SDMA engines
                    r0 = cc * 4 * DIM
                    nc.gpsimd.dma_start(
                        out=xins[cc % NXBUF][:, 0:XW].rearrange(
                            "j (q w) -> j q w", q=4),
                        in_=xt_d[r0:r0 + 4 * DIM, :].rearrange(
                            "(q j) w -> j q w", q=4))

                def slot_bd(ps, sbd, mov):
                    # per-item 30x30 products, 4 items per 128x128
                    # block-diagonal stationary (FWL-eligible 128-col
                    # loads); zeros outside the blocks keep pad lanes clean
                    for s in range(SLOTS):
                        nc.tensor.matmul(
                            ps[:, KP * s:KP * s + KP],
                            sbd[:, 128 * s:128 * (s + 1)],
                            mov[:, KP * s:KP * s + KP],
                            start=True, stop=True)

                def emit_A_g(c, g, st):
                    xin = xins[c % NXBUF]
                    psA = psA_pool.tile([128, SW], f32, tag="psA",
                                        name=f"psA{c}_{g}")
                    for i in range(SLOTS):
                        bl = g * SLOTS + i
                        nc.tensor.matmul(
                            psA[:, i * KP:(i + 1) * KP],
                            xin[:, bl * DIM:bl * DIM + 128],
                            wt_sb[:],
                            start=True, stop=True)
                    hsb = hpool.tile([DIM, SW], f16, tag="hsb",
                                     name=f"h{c}_{g}")
                    if g % 2 == 0:
                        nc.vector.tensor_copy(out=hsb[:], in_=psA[0:DIM, :])
                    else:
                        nc.scalar.copy(out=hsb[:], in_=psA[0:DIM, :])
                    st[f'h{g}'] = hsb

                def emit_p2(c, st):
                    psS = psS_pool.tile([128, SW], f32, tag="psS",
                                        name=f"psS{c}")
                    for r in range(4):
                        for g in range(4):
                            hsb3 = st[f'h{g}'][:].rearrange(
                                "z (i q) -> z i q", i=SLOTS)
                            nc.tensor.matmul(
                                psS[32 * r:32 * r + KP,
                                    g * 128:(g + 1) * 128],
                                wt_sb[:],
                                hsb3[:, r::4, :],
                                start=True, stop=True,
                                tile_position=(0, 32 * r),
                                skip_group_check=True)
                    st['psS'] = psS

                def emit_S0(c, st):
                    # u1 = 2s = 2*alpha*y + 2*beta*I (stacked layout),
                    # then scatter dense u1 into its block-diagonal
                    # stationary; the 4 strided copies are split across
                    # GpSimd/Scalar/DVE to balance engine load
                    u1 = upool.tile([128, SW], f16, tag="u1",
                                    name=f"u1_{c}")
                    nc.vector.tensor_add(u1[:], st['psS'][:], bet2_sb[:])
                    sbd1 = sbd1s[c % 2]
                    u13 = u1[:].rearrange("z (s q) -> z s q", s=SLOTS)
                    sbd13 = sbd1[:].rearrange("z (s w) -> z s w", s=SLOTS)
                    for r in range(4):
                        p0 = 32 * r
                        if r >= 2:
                            nc.vector.tensor_copy(
                                out=sbd13[p0:p0 + K, :, p0:p0 + K],
                                in_=u13[p0:p0 + K, :, 0:K])
                        else:
                            nc.scalar.copy(
                                out=sbd13[p0:p0 + K, :, p0:p0 + K],
                                in_=u13[p0:p0 + K, :, 0:K])
                    st['u1'], st['sbd1'] = u1, sbd1

                def emit_S1(c, st):
                    ps2 = psB_pool.tile([128, SW], f32, tag="psB",
                                        name=f"ps2_{c}")
                    slot_bd(ps2, st['sbd1'], st['u1'])      # 4s^2
                    u2 = upool.tile([128, SW], f16, tag="u2",
                                    name=f"u2_{c}")
                    nc.vector.tensor_sub(u2[:], ps2[:], idp2_sb[:])  # 2T2
                    st['u2'] = u2

                def emit_S2(c, st):
                    ps3 = psB_pool.tile([128, SW], f32, tag="psB",
                                        name=f"ps3_{c}")
                    slot_bd(ps3, st['sbd1'], st['u2'])      # 4sT2
                    u3 = upool.tile([128, SW], f16, tag="u3",
                                    name=f"u3_{c}")
                    nc.vector.tensor_sub(u3[:], ps3[:], st['u1'][:])  # 2T3
                    # scatter dense u3 into its block-diagonal stationary
                    # on the Scalar engine
                    sbd3 = sbd3s[c % 3]
                    u33 = u3[:].rearrange("z (s q) -> z s q", s=SLOTS)
                    sbd33 = sbd3[:].rearrange("z (s w) -> z s w", s=SLOTS)
                    for r in range(4):
                        p0 = 32 * r
                        nc.scalar.copy(
                            out=sbd33[p0:p0 + K, :, p0:p0 + K],
                            in_=u33[p0:p0 + K, :, 0:K])
                    st['u3'], st['sbd3'] = u3, sbd3

                def qconst(ps, movs, j0, start):
                    # ps (+)= sum_k CDEV[j0+k] * movs[k] via c*I stationaries
                    for k, mv in enumerate(movs):
                        nc.tensor.matmul(
                            ps[:],
                            cd_sb[:, (j0 + k) * 128:(j0 + k + 1) * 128],
                            mv[:],
                            start=(start and k == 0), stop=False,
                            skip_group_check=True)

                def emit_S3(c, st):
                    # A3 = c0*2I + c1*u1 + c2*u2 + c3*u3 on the PE,
                    # evacuated by the Scalar engine
                    psA3 = psB_pool.tile([128, SW], f32, tag="psB",
                                         name=f"psA3_{c}")
                    movs = [idp2_sb, st['u1'], st['u2'], st['u3']]
                    for k, mv in enumerate(movs):
                        nc.tensor.matmul(
                            psA3[:],
                            cd_sb[:, k * 128:(k + 1) * 128],
                            mv[:],
                            start=(k == 0), stop=(k == 3),
                            skip_group_check=True)
                    A3 = upool.tile([128, SW], f16, tag="A3",
                                    name=f"A3_{c}")
                    nc.vector.tensor_copy(out=A3[:], in_=psA3[:])
                    st['A3'] = A3

                def slot_bd_acc(ps, sbd, mov):
                    # like slot_bd but continuing an open accumulation group
                    for s in range(SLOTS):
                        nc.tensor.matmul(
                            ps[:, KP * s:KP * s + KP],
                            sbd[:, 128 * s:128 * (s + 1)],
                            mov[:, KP * s:KP * s + KP],
                            start=False, stop=(s == SLOTS - 1),
                            skip_group_check=True)

                def emit_S4(c, st):
                    # psq1 = c4*2I + c5*u1 + c6*u2 + 2T3*A3 in one PSUM
                    # accumulation group; A2 = 0.5*psq1 via Scalar
                    psq1 = psQ_pool.tile([128, SW], f32, tag="psq",
                                         name=f"psq1_{c}")
                    qconst(psq1, [idp2_sb, st['u1'], st['u2']], 4, True)
                    slot_bd_acc(psq1, st['sbd3'], st['A3'])
                    A2 = upool.tile([128, SW], f16, tag="A2",
                                    name=f"A2_{c}")
                    nc.vector.tensor_scalar_mul(A2[:], psq1[:], 0.5)
                    st['A2'] = A2

                def emit_S5(c, st):
                    # psq0 = c7*2I + c8*u1 + c9*u2 + 2T3*A2; evacuate with
                    # the 0.5 scale and the p-major relayout fused into one
                    # strided Scalar op
                    psq0 = psQ_pool.tile([128, SW], f32, tag="psq",
                                         name=f"psq0_{c}")
                    qconst(psq0, [st['u1'], st['u2']], 8, True)
                    slot_bd_acc(psq0, st['sbd3'], st['A2'])
                    lg3v = lg3[:].rearrange("z (p n) -> z p n", p=K)
                    psq0v = psq0[:].rearrange("z (s p) -> z p s", s=SLOTS)
                    nc.scalar.mul(
                        out=lg3v[:, :, c * SLOTS:(c + 1) * SLOTS],
                        in_=psq0v[:, 0:K, :], mul=0.5)

                qlen = nchunk // 2        # chunks per output half
                qcol = qlen * SLOTS       # (cc,s) columns per half

                ncol = nchunk * SLOTS

                def emit_C(qt):
                    # linear layer for batch quarter qt: one accumulation
                    # chain per item-group r, each in its OWN psC tile —
                    # multiple chains inside one PSUM tile wedge the HW.
                    qb = 4 * qcol
                    for r in range(4):
                        psC = psC_pool.tile([128, qcol], f32, tag="psC",
                                            name=f"psC_{qt}_{r}")
                        for p in range(K):
                            nc.tensor.matmul(
                                psC[:, :],
                                lw_sb[32 * r:32 * r + K,
                                      p * 128:(p + 1) * 128],
                                lg3[32 * r:32 * r + K,
                                    p * ncol + qt * qcol:
                                    p * ncol + (qt + 1) * qcol],
                                start=(p == 0), stop=(p == K - 1),
                                tile_position=(32 * r, 0),
                                skip_group_check=True)
                        nc.scalar.copy(
                            out=outsb[:, qb * qt + r:qb * (qt + 1):4],
                            in_=psC[0:CLS, :])
                    nc.gpsimd.dma_start(
                        out=out_d[:, qb * qt:qb * (qt + 1)],
                        in_=outsb[:, qb * qt:qb * (qt + 1)])

                # ---- depth-3 software pipeline:
                #   A-phase(t) | S-early(t-1) | S-late(t-2)
                # so each engine's FIFO always holds ready work while a
                # chain stage waits on another engine
                for cc in range(min(2, nchunk)):
                    fetch_x(cc)
                load_consts()
                for cc in range(2, min(4, nchunk)):
                    fetch_x(cc)
                states = {}
                for t in range(nchunk + 3):
                    a, b, c, d = t, t - 1, t - 2, t - 3
                    if a < nchunk:
                        if a + 4 < nchunk:
                            fetch_x(a + 4)
                        st_a = states[a] = {}
                        emit_A_g(a, 0, st_a)
                        emit_A_g(a, 1, st_a)
                        emit_A_g(a, 2, st_a)
                        emit_A_g(a, 3, st_a)
                        emit_p2(a, st_a)
                        emit_S0(a, st_a)
                    if 0 <= b < nchunk:
                        emit_S1(b, states[b])
                        emit_S2(b, states[b])
                        emit_S3(b, states[b])
                    if 0 <= c < nchunk:
                        emit_S4(c, states[c])
                    if 0 <= d < nchunk:
                        emit_S5(d, states[d])
                        del states[d]
                        if (d + 1) % qlen == 0:
                            emit_C((d + 1) // qlen - 1)

    nc.finalize()

    # ------------- host-side input prep
    wt_np, idp2_np, bet2_np, cd_np, qc_np, lw_np = _host_consts(
        W, lin_w, alpha, beta2 / 2.0)

    nchunk_h = bpc // CHUNK
    in_maps = []
    for ci in range(NCORES):
        xc = x[ci * bpc:(ci + 1) * bpc].astype(np.float16)  # [bpc, 93, 93]
        xtc = np.ascontiguousarray(
            xc.reshape(nchunk_h, 4, 16, DIM, DIM).transpose(0, 1, 3, 2, 4)
        ).reshape(nchunk_h * 4 * DIM, QUART)
        in_maps.append({"xt": xtc, "wt": wt_np, "idp2": idp2_np,
                        "bet2": bet2_np, "cd": cd_np, "qc": qc_np,
                        "lw": lw_np})

    res = run_bass_kernel_spmd(
        nc, in_maps, list(range(NCORES)),
        trace=bool(os.environ.get("BASS_TRACE")),
    )
    global LAST_EXEC_TIME_NS
    LAST_EXEC_TIME_NS = res.exec_time_ns
    outs = [res.results[i]["out"] for i in range(NCORES)]  # [117, bpc] each
    return np.concatenate([o.T for o in outs], axis=0)     # [B, 117]


def kernel(x, W, lin_w, lin_b):
    x = np.asarray(x, dtype=np.float32).reshape(B, DIM, DIM)
    W = np.asarray(W, dtype=np.float32)
    lin_w = np.asarray(lin_w, dtype=np.float32)
    lin_b = np.asarray(lin_b, dtype=np.float32)

    out = _run(x, W, lin_w, B // NCORES)
    # q0's constant term (CDEV[7] * I in the logm) applied via the bias:
    # logm += c7*I  =>  out += c7 * sum_q lin_w[:, q*30+q]
    idvec = np.zeros(900, np.float64)
    idvec[::31] = 1.0
    bias_c = (CDEV[7] * (lin_w.astype(np.float64) @ idvec)).astype(np.float32)
    return (out + (lin_b + bias_c)[None, :]).astype(np.float32)


# revision 59
# speedup vs baseline: 1.0145x; 1.0145x over previous
"""HDMNet (BiMap -> LogEig -> Linear) Trainium2 kernel, 8-core data-parallel.

Math: y_b = W^T x_b W (30x30 SPD), logm(y_b) approximated by a degree-9
Chebyshev polynomial in s = alpha*y + beta*I evaluated with a
Paterson-Stockmeyer block scheme:
    p(s) = q0(s) + T3'(s)*q1(s) + T3'(s)^2*q2'(s)
Per-item 30x30 products run as block-diagonal matmuls (4 items per
128x128 FWL-eligible stationary, built by strided scatter copies on
Scalar/DVE); scalar-coefficient combinations accumulate into the same
PSUM groups as c*I-stationary matmuls; q0's constant term is folded
into the host-side bias.

Layout: items are packed 4-per-partition-group (row groups 32r) and
32-columns-per-slot (30 data + 2 zero pad), so every PSUM region written
is a full 32-row/32-col block and the zero pads propagate exactly
(seeded by the zero-padded W stationary).  One item matrix lives at
[32r : 32r+30, 32s : 32s+30].

BiMap phase 1 uses per-item x-stationaries sliced 128 columns wide so
fast-weight-load engages (~27ns vs 80ns per load); the overhanging 35
columns read the next item's data / zero pad and land in PSUM rows
93..127, which are never read.  The linear layer interleaves its four
row-group accumulation chains across four separate PSUM tiles (chains
must never interleave inside one tile - that wedges the HW).

Dataflow: depth-4 software pipeline (A-phase(t) | T-recurrence(t-1) |
first Horner level(t-2) | second level + output(t-3)) so every engine
FIFO holds ready work; x arrives via SWDGE DMA (spreads across all 16
SDMA engines), with chunk 0/1 quartered onto the HWDGE rings for a fast
start.

Sharding: batch 8192 -> 1024 per NeuronCore; W / lin_w replicated.
"""
import os
import numpy as np

NCORES = 8
B = 8192
DIM, K, CLS = 93, 30, 117
KP = 32                     # padded slot pitch
CHUNK = 64
SLOTS = CHUNK // 4          # 16 slots of 4 stacked items
SW = SLOTS * KP             # 512 cols per chunk-state tile (1 PSUM bank)
QUART = 16 * 93             # 1488 cols per chunk-quarter DMA slab
A_LO, A_HI = 0.076, 3.51

# Chebyshev-basis constants, order:
# [(2,0)..(2,3), (1,0)..(1,2), (0,0)..(0,2)]  (level i, Cheb index k)
CDEV = [
    -0.015716552734375,
    0.0946044921875,
    0.0018758773803710938,
    0.0304412841796875,
    0.255859375,
    -0.202880859375,
    0.131591796875,
    0.18896484375,
    1.35546875,
    -0.395751953125,
]

LAST_EXEC_TIME_NS = None


def _host_consts(W, lin_w, alpha, beta):
    f16 = np.float16
    wt = np.zeros((DIM, KP), f16)
    wt[:, :K] = (np.sqrt(2.0 * alpha) * W).astype(f16)          # [93,32]

    # stacked identity pattern [128, SW]: 2*I at each (group, slot)
    idp2 = np.zeros((128, SW), np.float32)
    eye2 = 2.0 * np.eye(K, dtype=np.float32)
    for r in range(4):
        for s in range(SLOTS):
            idp2[32 * r:32 * r + K, KP * s:KP * s + K] = eye2
    bet2 = (beta * idp2).astype(np.float32)                     # 2*beta*I stacked
    idp2_16 = idp2.astype(f16)

    # constant-diagonal stationaries [128, 10*128] for the coefficient
    # matmuls (movings are [idp2, u1, u2] per level); index 7 (q0's
    # constant term) is unused on-device -- it is folded into the host
    # bias as CDEV[7]*lin_w@vec(I)
    cd = np.zeros((128, 10 * 128), f16)
    i128 = np.eye(128, dtype=np.float32)
    for j, c in enumerate(CDEV):
        cd[:, j * 128:(j + 1) * 128] = (c * i128).astype(f16)
    qc = (CDEV[0] * idp2).astype(f16)            # A3 chain seed

    # linear weights banked, CLS padded to 128: lw[32r+q, p*128+cls]
    lw = np.zeros((128, K * 128), f16)
    lwr = lin_w.reshape(CLS, K, K)          # [cls, p, q]
    blk = np.zeros((K, K * 128), np.float32)
    for p in range(K):
        blk[:, p * 128:p * 128 + CLS] = lwr[:, p, :].T          # [q, cls]
    for r in range(4):
        lw[32 * r:32 * r + K, :] = blk.astype(f16)
    return wt, idp2_16, bet2, cd, qc, lw


def _run(x, W, lin_w, bpc):
    import concourse.bass as bass
    import concourse.bacc as bacc
    import concourse.mybir as mybir
    from concourse.tile import TileContext
    from concourse.bass_utils import run_bass_kernel_spmd

    f16, f32 = mybir.dt.float16, mybir.dt.float32
    MULT, ADD = mybir.AluOpType.mult, mybir.AluOpType.add
    nchunk = bpc // CHUNK
    alpha = 2.0 / (A_HI - A_LO)
    beta2 = -2.0 * (A_HI + A_LO) / (A_HI - A_LO)   # 2*beta

    nc = bacc.Bacc()
    xt_d = nc.dram_tensor("xt", [nchunk * 4 * DIM, QUART], f16,
                          kind="ExternalInput")
    wt_d = nc.dram_tensor("wt", [DIM, KP], f16, kind="ExternalInput")
    idp2_d = nc.dram_tensor("idp2", [128, SW], f16, kind="ExternalInput")
    bet2_d = nc.dram_tensor("bet2", [128, SW], f32, kind="ExternalInput")
    cd_d = nc.dram_tensor("cd", [128, 10 * 128], f16, kind="ExternalInput")
    qc_d = nc.dram_tensor("qc", [128, SW], f16, kind="ExternalInput")
    lw_d = nc.dram_tensor("lw", [128, K * 128], f16, kind="ExternalInput")
    out_d = nc.dram_tensor("out", [CLS, bpc], f32, kind="ExternalOutput")

    XW = CHUNK * DIM           # 5952 data cols per xin tile
    XPAD = XW + 35             # room for the last item's 128-col slice

    with TileContext(nc) as tc:
        with tc.sbuf_pool(name="cpool", bufs=1) as cpool, \
             tc.sbuf_pool(name="xpool", bufs=1) as xpool, \
             tc.sbuf_pool(name="hpool", bufs=4) as hpool, \
             tc.sbuf_pool(name="upool", bufs=3) as upool, \
             tc.sbuf_pool(name="spool", bufs=1) as spool:

            wt_sb = cpool.tile([DIM, KP], f16, name="wt_sb")
            nc.sync.dma_start(out=wt_sb[:], in_=wt_d[:])
            idp2_sb = cpool.tile([128, SW], f16, name="idp2_sb")
            bet2_sb = cpool.tile([128, SW], f32, name="bet2_sb")
            cd_sb = cpool.tile([128, 10 * 128], f16, name="cd_sb")
            qc_sb = cpool.tile([128, SW], f16, name="qc_sb")
            lw_sb = cpool.tile([128, K * 128], f16, name="lw_sb")

            def load_consts():
                # small consts on the scalar ring; big ones (cd/lw) via
                # SWDGE, drained after the startup chunks
                nc.scalar.dma_start(out=bet2_sb[:], in_=bet2_d[:])
                nc.scalar.dma_start(out=idp2_sb[:], in_=idp2_d[:])
                nc.scalar.dma_start(out=qc_sb[:], in_=qc_d[:])
                nc.gpsimd.dma_start(out=cd_sb[:], in_=cd_d[:])
                nc.gpsimd.dma_start(out=lw_sb[:], in_=lw_d[:])

            NXBUF = 5
            xins = [xpool.tile([DIM, XPAD], f16, tag=f"xin{i}",
                               name=f"xin{i}") for i in range(NXBUF)]
            for t in xins:
                nc.vector.memset(t[:, XW:], 0.0)

            # p-major: lg3[z, p*(nchunk*SLOTS) + cc*SLOTS + s]
            # block-diagonal stationaries (zeros persist outside the
            # 30x30 data blocks; scatters rewrite only those)
            sbd1s = [spool.tile([128, SLOTS * 128], f16, name=f"sbd1_{i}")
                     for i in range(2)]
            sbd3s = [spool.tile([128, SLOTS * 128], f16, name=f"sbd3_{i}")
                     for i in range(3)]
            for i, tl in enumerate(sbd1s + sbd3s):
                (nc.vector if i % 2 == 0 else nc.gpsimd).memset(tl[:], 0.0)

            lg3 = spool.tile([128, K * nchunk * SLOTS], f16, name="lg3")
            outsb = spool.tile([CLS, bpc], f32, name="outsb")

            with tc.psum_pool(name="psA", bufs=2) as psA_pool, \
                 tc.psum_pool(name="psS", bufs=1) as psS_pool, \
                 tc.psum_pool(name="psB", bufs=2) as psB_pool, \
                 tc.psum_pool(name="psQ", bufs=2) as psQ_pool, \
                 tc.psum_pool(name="psC", bufs=1) as psC_pool:

                def fetch_x(cc):
                    if cc < 2:
                        # startup: quarter-DMAs on the idle HWDGE rings so
                        # chunk 0's first slots are usable within ~10us
                        for q in range(4):
                            eng = nc.sync if cc == 0 else nc.scalar
                            r0 = (cc * 4 + q) * DIM
                            eng.dma_start(
                                out=xins[cc][:, q * QUART:(q + 1) * QUART],
                                in_=xt_d[r0:r0 + DIM, :])
                        return
                    # steady state via SWDGE (gpsimd): spreads each
                    # transfer across all 16 SDMA engines
                    r0 = cc * 4 * DIM
                    nc.gpsimd.dma_start(
                        out=xins[cc % NXBUF][:, 0:XW].rearrange(
                            "j (q w) -> j q w", q=4),
                        in_=xt_d[r0:r0 + 4 * DIM, :].rearrange(
                            "(q j) w -> j q w", q=4))

                def slot_bd(ps, sbd, mov):
                    # per-item 30x30 products, 4 items per 128x128
                    # block-diagonal stationary (FWL-eligible 128-col
                    # loads); zeros outside the blocks keep pad lanes clean
                    for s in range(SLOTS):
                        nc.tensor.matmul(
                            ps[:, KP * s:KP * s + KP],
                            sbd[:, 128 * s:128 * (s + 1)],
                            mov[:, KP * s:KP * s + KP],
                            start=True, stop=True)

                def emit_A_g(c, g, st):
                    xin = xins[c % NXBUF]
                    psA = psA_pool.tile([128, SW], f32, tag="psA",
                                        name=f"psA{c}_{g}")
                    for i in range(SLOTS):
                        bl = g * SLOTS + i
                        nc.tensor.matmul(
                            psA[:, i * KP:(i + 1) * KP],
                            xin[:, bl * DIM:bl * DIM + 128],
                            wt_sb[:],
                            start=True, stop=True)
                    hsb = hpool.tile([DIM, SW], f16, tag="hsb",
                                     name=f"h{c}_{g}")
                    if g % 2 == 0:
                        nc.vector.tensor_copy(out=hsb[:], in_=psA[0:DIM, :])
                    else:
                        nc.scalar.copy(out=hsb[:], in_=psA[0:DIM, :])
                    st[f'h{g}'] = hsb

                def emit_p2(c, st):
                    psS = psS_pool.tile([128, SW], f32, tag="psS",
                                        name=f"psS{c}")
                    for r in range(4):
                        for g in range(4):
                            hsb3 = st[f'h{g}'][:].rearrange(
                                "z (i q) -> z i q", i=SLOTS)
                            nc.tensor.matmul(
                                psS[32 * r:32 * r + KP,
                                    g * 128:(g + 1) * 128],
                                wt_sb[:],
                                hsb3[:, r::4, :],
                                start=True, stop=True,
                                tile_position=(0, 32 * r),
                                skip_group_check=True)
                    st['psS'] = psS

                def emit_S0(c, st):
                    # u1 = 2s = 2*alpha*y + 2*beta*I (stacked layout),
                    # then scatter dense u1 into its block-diagonal
                    # stationary; the 4 strided copies are split across
                    # GpSimd/Scalar/DVE to balance engine load
                    u1 = upool.tile([128, SW], f16, tag="u1",
                                    name=f"u1_{c}")
                    nc.vector.tensor_add(u1[:], st['psS'][:], bet2_sb[:])
                    sbd1 = sbd1s[c % 2]
                    u13 = u1[:].rearrange("z (s q) -> z s q", s=SLOTS)
                    sbd13 = sbd1[:].rearrange("z (s w) -> z s w", s=SLOTS)
                    for r in range(4):
                        p0 = 32 * r
                        if r >= 2:
                            nc.vector.tensor_copy(
                                out=sbd13[p0:p0 + K, :, p0:p0 + K],
                                in_=u13[p0:p0 + K, :, 0:K])
                        else:
                            nc.scalar.copy(
                                out=sbd13[p0:p0 + K, :, p0:p0 + K],
                                in_=u13[p0:p0 + K, :, 0:K])
                    st['u1'], st['sbd1'] = u1, sbd1

                def emit_S1(c, st):
                    ps2 = psB_pool.tile([128, SW], f32, tag="psB",
                                        name=f"ps2_{c}")
                    slot_bd(ps2, st['sbd1'], st['u1'])      # 4s^2
                    u2 = upool.tile([128, SW], f16, tag="u2",
                                    name=f"u2_{c}")
                    nc.vector.tensor_sub(u2[:], ps2[:], idp2_sb[:])  # 2T2
                    st['u2'] = u2

                def emit_S2(c, st):
                    ps3 = psB_pool.tile([128, SW], f32, tag="psB",
                                        name=f"ps3_{c}")
                    slot_bd(ps3, st['sbd1'], st['u2'])      # 4sT2
                    u3 = upool.tile([128, SW], f16, tag="u3",
                                    name=f"u3_{c}")
                    nc.vector.tensor_sub(u3[:], ps3[:], st['u1'][:])  # 2T3
                    # scatter dense u3 into its block-diagonal stationary
                    # on the Scalar engine
                    sbd3 = sbd3s[c % 3]
                    u33 = u3[:].rearrange("z (s q) -> z s q", s=SLOTS)
                    sbd33 = sbd3[:].rearrange("z (s w) -> z s w", s=SLOTS)
                    for r in range(4):
                        p0 = 32 * r
                        nc.scalar.copy(
                            out=sbd33[p0:p0 + K, :, p0:p0 + K],
                            in_=u33[p0:p0 + K, :, 0:K])
                    st['u3'], st['sbd3'] = u3, sbd3

                def qconst(ps, movs, j0, start):
                    # ps (+)= sum_k CDEV[j0+k] * movs[k] via c*I stationaries
                    for k, mv in enumerate(movs):
                        nc.tensor.matmul(
                            ps[:],
                            cd_sb[:, (j0 + k) * 128:(j0 + k + 1) * 128],
                            mv[:],
                            start=(start and k == 0), stop=False,
                            skip_group_check=True)

                def emit_S3(c, st):
                    # A3 = c0*2I + c1*u1 + c2*u2 + c3*u3 on the PE,
                    # evacuated by the Scalar engine
                    psA3 = psB_pool.tile([128, SW], f32, tag="psB",
                                         name=f"psA3_{c}")
                    movs = [idp2_sb, st['u1'], st['u2'], st['u3']]
                    for k, mv in enumerate(movs):
                        nc.tensor.matmul(
                            psA3[:],
                            cd_sb[:, k * 128:(k + 1) * 128],
                            mv[:],
                            start=(k == 0), stop=(k == 3),
                            skip_group_check=True)
                    A3 = upool.tile([128, SW], f16, tag="A3",
                                    name=f"A3_{c}")
                    nc.vector.tensor_copy(out=A3[:], in_=psA3[:])
                    st['A3'] = A3

                def slot_bd_acc(ps, sbd, mov):
                    # like slot_bd but continuing an open accumulation group
                    for s in range(SLOTS):
                        nc.tensor.matmul(
                            ps[:, KP * s:KP * s + KP],
                            sbd[:, 128 * s:128 * (s + 1)],
                            mov[:, KP * s:KP * s + KP],
                            start=False, stop=(s == SLOTS - 1),
                            skip_group_check=True)

                def emit_S4(c, st):
                    # psq1 = c4*2I + c5*u1 + c6*u2 + 2T3*A3 in one PSUM
                    # accumulation group; A2 = 0.5*psq1 via Scalar
                    psq1 = psQ_pool.tile([128, SW], f32, tag="psq",
                                         name=f"psq1_{c}")
                    qconst(psq1, [idp2_sb, st['u1'], st['u2']], 4, True)
                    slot_bd_acc(psq1, st['sbd3'], st['A3'])
                    A2 = upool.tile([128, SW], f16, tag="A2",
                                    name=f"A2_{c}")
                    nc.vector.tensor_scalar_mul(A2[:], psq1[:], 0.5)
                    st['A2'] = A2

                def emit_S5(c, st):
                    # psq0 = c7*2I + c8*u1 + c9*u2 + 2T3*A2; evacuate with
                    # the 0.5 scale and the p-major relayout fused into one
                    # strided Scalar op
                    psq0 = psQ_pool.tile([128, SW], f32, tag="psq",
                                         name=f"psq0_{c}")
                    qconst(psq0, [st['u1'], st['u2']], 8, True)
                    slot_bd_acc(psq0, st['sbd3'], st['A2'])
                    lg3v = lg3[:].rearrange("z (p n) -> z p n", p=K)
                    psq0v = psq0[:].rearrange("z (s p) -> z p s", s=SLOTS)
                    nc.scalar.mul(
                        out=lg3v[:, :, c * SLOTS:(c + 1) * SLOTS],
                        in_=psq0v[:, 0:K, :], mul=0.5)

                qlen = nchunk // 4        # chunks per output quarter
                qcol = qlen * SLOTS       # (cc,s) columns per quarter

                ncol = nchunk * SLOTS

                def emit_C(qt):
                    # linear layer for batch quarter qt: one accumulation
                    # chain per item-group r, each in its OWN psC tile —
                    # multiple chains inside one PSUM tile wedge the HW.
                    qb = 4 * qcol
                    for r in range(4):
                        psC = psC_pool.tile([128, qcol], f32, tag="psC",
                                            name=f"psC_{qt}_{r}")
                        for p in range(K):
                            nc.tensor.matmul(
                                psC[:, :],
                                lw_sb[32 * r:32 * r + K,
                                      p * 128:(p + 1) * 128],
                                lg3[32 * r:32 * r + K,
                                    p * ncol + qt * qcol:
                                    p * ncol + (qt + 1) * qcol],
                                start=(p == 0), stop=(p == K - 1),
                                tile_position=(32 * r, 0),
                                skip_group_check=True)
                        nc.scalar.copy(
                            out=outsb[:, qb * qt + r:qb * (qt + 1):4],
                            in_=psC[0:CLS, :])
                    nc.gpsimd.dma_start(
                        out=out_d[:, qb * qt:qb * (qt + 1)],
                        in_=outsb[:, qb * qt:qb * (qt + 1)])

                # ---- depth-3 software pipeline:
                #   A-phase(t) | S-early(t-1) | S-late(t-2)
                # so each engine's FIFO always holds ready work while a
                # chain stage waits on another engine
                for cc in range(min(2, nchunk)):
                    fetch_x(cc)
                load_consts()
                for cc in range(2, min(4, nchunk)):
                    fetch_x(cc)
                states = {}
                for t in range(nchunk + 3):
                    a, b, c, d = t, t - 1, t - 2, t - 3
                    if a < nchunk:
                        if a + 4 < nchunk:
                            fetch_x(a + 4)
                        st_a = states[a] = {}
                        emit_A_g(a, 0, st_a)
                        emit_A_g(a, 1, st_a)
                        emit_A_g(a, 2, st_a)
                        emit_A_g(a, 3, st_a)
                        emit_p2(a, st_a)
                        emit_S0(a, st_a)
                    if 0 <= b < nchunk:
                        emit_S1(b, states[b])
                        emit_S2(b, states[b])
                        emit_S3(b, states[b])
                    if 0 <= c < nchunk:
                        emit_S4(c, states[c])
                    if 0 <= d < nchunk:
                        emit_S5(d, states[d])
                        del states[d]
                        if (d + 1) % qlen == 0:
                            emit_C((d + 1) // qlen - 1)

    nc.finalize()

    # ------------- host-side input prep
    wt_np, idp2_np, bet2_np, cd_np, qc_np, lw_np = _host_consts(
        W, lin_w, alpha, beta2 / 2.0)

    nchunk_h = bpc // CHUNK
    in_maps = []
    for ci in range(NCORES):
        xc = x[ci * bpc:(ci + 1) * bpc].astype(np.float16)  # [bpc, 93, 93]
        xtc = np.ascontiguousarray(
            xc.reshape(nchunk_h, 4, 16, DIM, DIM).transpose(0, 1, 3, 2, 4)
        ).reshape(nchunk_h * 4 * DIM, QUART)
        in_maps.append({"xt": xtc, "wt": wt_np, "idp2": idp2_np,
                        "bet2": bet2_np, "cd": cd_np, "qc": qc_np,
                        "lw": lw_np})

    res = run_bass_kernel_spmd(
        nc, in_maps, list(range(NCORES)),
        trace=bool(os.environ.get("BASS_TRACE")),
    )
    global LAST_EXEC_TIME_NS
    LAST_EXEC_TIME_NS = res.exec_time_ns
    outs = [res.results[i]["out"] for i in range(NCORES)]  # [117, bpc] each
    return np.concatenate([o.T for o in outs], axis=0)     # [B, 117]


def kernel(x, W, lin_w, lin_b):
    x = np.asarray(x, dtype=np.float32).reshape(B, DIM, DIM)
    W = np.asarray(W, dtype=np.float32)
    lin_w = np.asarray(lin_w, dtype=np.float32)
    lin_b = np.asarray(lin_b, dtype=np.float32)

    out = _run(x, W, lin_w, B // NCORES)
    # q0's constant term (CDEV[7] * I in the logm) applied via the bias:
    # logm += c7*I  =>  out += c7 * sum_q lin_w[:, q*30+q]
    idvec = np.zeros(900, np.float64)
    idvec[::31] = 1.0
    bias_c = (CDEV[7] * (lin_w.astype(np.float64) @ idvec)).astype(np.float32)
    return (out + (lin_b + bias_c)[None, :]).astype(np.float32)
